# revision 25
# baseline (speedup 1.0000x reference)
"""AxialAttention Trainium2 kernel.

Reference computation (per batch element b of 516, heads H=8, L=M=129,
DK=64, DV=128, C=512):
  qkv = x @ Wqkv, BN -> q,k,v
  sims = content(q,k) + queryRPE(q,qt) + keyRPE(k,kt)  (per-component BN scale,
         shift dropped: softmax-invariant)
  w = softmax(sims)
  out = BN0(w @ v) + BN1(w @ vr)   (vr = skewed value RPE table)

Strategy: data-parallel over batch across 8 NeuronCores (65 padded batches
per core).  All BN affine transforms are folded on the host into the qkv
weights / RPE tables.  On-chip per batch: x -> xT (PE transpose),
qkT = W.T @ x.T GEMM, v natural GEMM, then per-head attention with the
relative-position skews done as strided SBUF->SBUF DMA gathers/scatters.
"""

import os
import numpy as np
import ml_dtypes

EPS = 1e-3
H, DK, DV = 8, 64, 128
B, L, C = 516, 129, 512
W2L = 2 * L - 1  # 257
NCORES = 8
NB = 65          # batches per core (65*8 = 520 >= 516)
GRP = 13         # batch group size (65 = 5*13); qk GEMM packs 3 within group
BF16 = ml_dtypes.bfloat16

_PROGRAM_CACHE = {}


def _affine(mean, var, gamma, beta):
    s = gamma / np.sqrt(var + EPS)
    t = beta - mean * s
    return s.astype(np.float32), t.astype(np.float32)


def _host_prep(inputs):
    """Fold BN into weights/tables; build device-layout arrays."""
    f32 = np.float32
    gq, bq = f32(inputs["gamma_qkv"]), f32(inputs["beta_qkv"])
    mq, vq = f32(inputs["mean_qkv"]), f32(inputs["var_qkv"])
    s_qkv, t_qkv = _affine(mq, vq, gq, bq)

    s_sim = (f32(inputs["gamma_sim"]) /
             np.sqrt(f32(inputs["var_sim"]) + EPS)).astype(f32)  # [3, H]
    s_out, t_out = _affine(f32(inputs["mean_out"]), f32(inputs["var_out"]),
                           f32(inputs["gamma_out"]), f32(inputs["beta_out"]))

    colscale = np.ones(2048, f32)
    colscale[512:1024] = np.repeat(s_sim[0], DK)          # k channels
    colscale[1024:2048] = s_out[0].reshape(-1)            # v channels
    Wf = (f32(inputs["qkv_kernel"]) * (s_qkv * colscale)[None, :]).astype(f32)
    tf = (t_qkv * colscale).astype(f32)

    qt = f32(inputs["query_rpe_table"])      # [257, 64]
    kt = f32(inputs["key_rpe_table"])        # [257, 64]
    vt = f32(inputs["value_rpe_table"])      # [257, 128]
    ratio = (s_sim[2] / s_sim[0]).astype(f32)             # [H]

    # tables transposed [64, ...] per head, duplicated into both partition
    # halves so lhsT/rhs base partitions match for odd heads.
    qtT = np.einsum('h,jd->hdj', s_sim[1], qt)            # [H, 64, 257]
    ktrT = np.einsum('h,jd->hdj', ratio, kt[::-1])        # [H, 64, 257]
    ktnT = np.einsum('h,jd->hdj', ratio, kt[0:L])         # [H, 64, 129]

    def dup(tab):  # [H, 64, N] -> [128, H, N]
        t = np.transpose(tab, (1, 0, 2))                  # [64, H, N]
        return np.concatenate([t, t], axis=0).copy()      # [128, H, N]

    arrs = {
        "Wb": np.ascontiguousarray(
            Wf.reshape(4, 128, 2048).transpose(1, 0, 2)).astype(BF16),
        "qtT": dup(qtT).astype(BF16),
        "ktrT": dup(ktrT).astype(BF16),
        "ktnT": dup(ktnT).astype(BF16),
        "vt0": np.ascontiguousarray(
            np.einsum('hd,jd->jhd', s_out[1], vt[0:128])).astype(BF16),
        "vt1": np.ascontiguousarray(
            np.einsum('hd,jd->jhd', s_out[1], vt[128:256])).astype(BF16),
        "vtrev": np.ascontiguousarray(
            np.einsum('hd,jd->jhd', s_out[1], vt[256:128:-1])).astype(BF16),
        "tq": np.ascontiguousarray(
            tf[0:1024].reshape(8, 128).T).astype(f32),     # [128, 8]
        "tv": tf[1024:2048].reshape(1, 1024).astype(f32),
        "tb": (t_out[0] + t_out[1]).reshape(1, 1024).astype(f32),
    }
    return arrs


def _build_program(nb):
    """Build the SPMD Bass program for nb batches."""
    import concourse.bass as bass
    import concourse.mybir as mybir
    import concourse.tile as tile
    from concourse import bacc
    from concourse.masks import make_identity

    fp32 = mybir.dt.float32
    bf16 = mybir.dt.bfloat16

    nc = bacc.Bacc("TRN2", target_bir_lowering=False, debug=False)
    X = nc.dram_tensor("x", [nb, L, C], bf16, kind="ExternalInput")
    Wb = nc.dram_tensor("Wb", [128, 4, 2048], bf16, kind="ExternalInput")
    QT = nc.dram_tensor("qtT", [128, H, W2L], bf16, kind="ExternalInput")
    KRT = nc.dram_tensor("ktrT", [128, H, W2L], bf16, kind="ExternalInput")
    KNT = nc.dram_tensor("ktnT", [128, H, L], bf16, kind="ExternalInput")
    VT0 = nc.dram_tensor("vt0", [128, H, DV], bf16, kind="ExternalInput")
    VT1 = nc.dram_tensor("vt1", [128, H, DV], bf16, kind="ExternalInput")
    VTR = nc.dram_tensor("vtrev", [128, H, DV], bf16, kind="ExternalInput")
    TQ = nc.dram_tensor("tq", [128, 8], fp32, kind="ExternalInput")
    TV = nc.dram_tensor("tv", [1, 1024], fp32, kind="ExternalInput")
    TB = nc.dram_tensor("tb", [1, 1024], fp32, kind="ExternalInput")
    OUT = nc.dram_tensor("out", [nb, L, 1024], fp32, kind="ExternalOutput")

    Exp = mybir.ActivationFunctionType.Exp

    with tile.TileContext(nc) as tc:
        with (
            tc.tile_pool(name="res", bufs=1) as res,
            tc.tile_pool(name="grp", bufs=2) as grp,
            tc.tile_pool(name="sb", bufs=2) as sb,
            tc.tile_pool(name="sb3", bufs=3) as sb3,
            tc.tile_pool(name="sb1", bufs=1) as sb1,
            tc.tile_pool(name="tiny", bufs=4) as tiny,
            tc.tile_pool(name="psG", bufs=2, space="PSUM") as psG,
            tc.tile_pool(name="ps128", bufs=2, space="PSUM") as ps128,
            tc.tile_pool(name="psS", bufs=1, space="PSUM") as psS,
            tc.tile_pool(name="psT", bufs=1, space="PSUM") as psT,
            tc.tile_pool(name="psR", bufs=1, space="PSUM") as psR,
        ):
            # ---- residents ----
            w_sb = res.tile([128, 4, 2048], bf16)
            nc.sync.dma_start(out=w_sb, in_=Wb[:, :, :])
            qtT = res.tile([128, H, W2L], bf16)
            nc.sync.dma_start(out=qtT, in_=QT[:, :, :])
            ktrT = res.tile([128, H, W2L], bf16)
            nc.sync.dma_start(out=ktrT, in_=KRT[:, :, :])
            ktnT = res.tile([128, H, L], bf16)
            nc.sync.dma_start(out=ktnT, in_=KNT[:, :, :])
            vt0 = res.tile([128, H, DV], bf16)
            nc.sync.dma_start(out=vt0, in_=VT0[:, :, :])
            vt1 = res.tile([128, H, DV], bf16)
            nc.sync.dma_start(out=vt1, in_=VT1[:, :, :])
            vtrev = res.tile([128, H, DV], bf16)
            nc.sync.dma_start(out=vtrev, in_=VTR[:, :, :])
            tq_sb = res.tile([128, 8], fp32)
            nc.sync.dma_start(out=tq_sb, in_=TQ[:, :])
            tv_sb = res.tile([1, 1024], fp32)
            nc.sync.dma_start(out=tv_sb, in_=TV[:, :])
            tb_sb = res.tile([1, 1024], fp32)
            nc.sync.dma_start(out=tb_sb, in_=TB[:, :])
            # partition-replicated copies (compute engines cannot broadcast
            # across partitions)
            tvb = res.tile([128, 1024], fp32)
            nc.gpsimd.dma_start(out=tvb, in_=TV[0:1, :].partition_broadcast(128))
            tbb = res.tile([128, 1024], fp32)
            nc.gpsimd.dma_start(out=tbb, in_=TB[0:1, :].partition_broadcast(128))
            id_bf = res.tile([128, 128], bf16)
            make_identity(nc, id_bf)
            id_f32 = res.tile([128, 128], fp32)
            make_identity(nc, id_f32)
            ones_sb = res.tile([128, 1], bf16)
            nc.vector.memset(ones_sb, 1.0)

            for g0 in range(0, nb, GRP):
                gn = min(GRP, nb - g0)
                # ---- phase 1: load x, transpose to xT for the group ----
                xt_grp = grp.tile([128, GRP, 4, L], bf16, tag="xt")
                xrem = grp.tile([128, 4, GRP], bf16, tag="xrem")
                for g in range(gn):
                    b = g0 + g
                    x0 = sb3.tile([128, C], bf16, tag="x0")
                    nc.sync.dma_start(out=x0, in_=X[b, 0:128, :])
                    x1 = tiny.tile([1, C], bf16, tag="x1")
                    nc.sync.dma_start(out=x1, in_=X[b, 128:129, :])
                    for cc in range(4):
                        pt = psT.tile([128, 128], bf16, tag="ptr")
                        nc.tensor.matmul(pt, x0[:, cc * 128:(cc + 1) * 128],
                                         id_bf, is_transpose=True,
                                         start=True, stop=True)
                        nc.vector.tensor_copy(out=xt_grp[:, g, cc, 0:128],
                                              in_=pt)
                        pt1 = psT.tile([128, 128], bf16, tag="ptr")
                        nc.tensor.matmul(pt1[:, 0:1],
                                         x1[0:1, cc * 128:(cc + 1) * 128],
                                         id_bf[0:1, 0:1], is_transpose=True,
                                         start=True, stop=True)
                        nc.vector.tensor_copy(out=xt_grp[:, g, cc, 128:129],
                                              in_=pt1[:, 0:1])
                        nc.vector.tensor_copy(out=xrem[:, cc, g:g + 1],
                                              in_=pt1[:, 0:1])

                # ---- group v-remainder GEMM: v1_all[g, 8, 128] ----
                v1_all = grp.tile([GRP, H, DV], bf16, tag="v1all")
                for half in range(2):
                    pv = psG.tile([GRP, 512], fp32, tag="pg")
                    for cc in range(4):
                        nc.tensor.matmul(
                            pv[0:gn, :], xrem[:, cc, 0:gn],
                            w_sb[:, cc, 1024 + half * 512:1536 + half * 512],
                            start=(cc == 0), stop=(cc == 3))
                    nc.vector.tensor_add(
                        out=v1_all[0:gn, half * 4:(half + 1) * 4, :]
                            .rearrange("g a b -> g (a b)"),
                        in0=pv[0:gn, :],
                        in1=tvb[0:gn, half * 512:(half + 1) * 512])

                # ---- qk GEMM with rhs packing of up to 3 batches ----
                qk_list = {}
                for p0 in range(0, gn, 3):
                    pn = min(3, gn - p0)
                    npack = pn * L
                    for g in range(p0, p0 + pn):
                        qk_list[g] = sb.tile([128, 8, L], bf16,
                                             name=f"qk{g % 3}",
                                             tag=f"qk{g % 3}")
                    for dc in range(8):
                        pqk = psG.tile([128, 3 * L], fp32, tag="pg")
                        for cc in range(4):
                            nc.tensor.matmul(
                                pqk[:, 0:npack],
                                w_sb[:, cc, dc * 128:(dc + 1) * 128],
                                xt_grp[:, p0:p0 + pn, cc, :],
                                start=(cc == 0), stop=(cc == 3))
                        for i in range(pn):
                            g = p0 + i
                            nc.vector.tensor_scalar_add(
                                out=qk_list[g][:, dc, :],
                                in0=pqk[:, i * L:(i + 1) * L],
                                scalar1=tq_sb[:, dc:dc + 1])

                    for i in range(pn):
                        g = p0 + i
                        b = g0 + g
                        qk_sb = qk_list[g]
                        _attention(nc, b, qk_sb, g, xt_grp,
                                   w_sb, qtT, ktrT, ktnT, vt0, vt1, vtrev,
                                   tvb, tbb, tb_sb, id_bf, id_f32, ones_sb,
                                   v1_all, sb, sb3, sb1, tiny,
                                   psG, ps128, psS, psT, psR,
                                   OUT, fp32, bf16, Exp)
    nc.finalize()
    return nc


def _attention(nc, b, qk_sb, g, xt_grp, w_sb, qtT, ktrT, ktnT, vt0, vt1,
               vtrev, tvb, tbb, tb_sb, id_bf, id_f32, ones_sb, v1_all,
               sb, sb3, sb1, tiny, psG, ps128, psS, psT, psR,
               OUT, fp32, bf16, Exp):
    """Attention for one batch: qk_sb [128, 8(dc), 129] bf16 in SBUF."""
    import concourse.bass as bass

    L_, H_, DV_ = L, H, DV

    # ---- v natural GEMM: v_sb [m=128, h, d] ----
    v_sb = sb.tile([128, H_, DV_], bf16, tag="vsb")
    for half in range(2):
        pv = psG.tile([128, 512], fp32, tag="pg")
        for cc in range(4):
            nc.tensor.matmul(
                pv, xt_grp[:, g, cc, 0:128],
                w_sb[:, cc, 1024 + half * 512:1536 + half * 512],
                start=(cc == 0), stop=(cc == 3))
        nc.vector.tensor_add(
            out=v_sb[:, half * 4:(half + 1) * 4, :]
                .rearrange("p a b -> p (a b)"),
            in0=pv,
            in1=tvb[:, half * 512:(half + 1) * 512])

    # v1 row of this batch -> partition 0
    v1b = tiny.tile([1, H_, DV_], bf16, tag="v1b")
    nc.sync.dma_start(out=v1b, in_=v1_all[g:g + 1, :, :])

    aq_all = sb1.tile([128, H_, W2L], fp32, tag="aq")
    bk_all = sb1.tile([128, H_, W2L], fp32, tag="bk")

    w1h_list = []
    w1t_sb = sb.tile([128, H_], bf16, tag="w1t")

    # ---- pass 1: RPE table matmuls for all heads ----
    for h in range(H_):
        r0 = (h % 2) * 64
        dcq, dck = h // 2, 4 + h // 2
        qTl = qk_sb[r0:r0 + 64, dcq, 0:128]      # lhsT [64, 128]
        ps_aq = ps128.tile([128, W2L], fp32, tag="tab")
        nc.tensor.matmul(ps_aq, qTl, qtT[r0:r0 + 64, h, :],
                         start=True, stop=True)
        nc.vector.tensor_copy(out=aq_all[:, h, :], in_=ps_aq)
        ps_bk = ps128.tile([128, W2L], fp32, tag="tab")
        nc.tensor.matmul(ps_bk, qk_sb[r0:r0 + 64, dck, 0:128],
                         ktrT[r0:r0 + 64, h, :], start=True, stop=True)
        nc.vector.tensor_copy(out=bk_all[:, h, :], in_=ps_bk)

    # ---- skew DMAs (all heads at once) ----
    sims2 = sb1.tile([128, H_, L_], fp32, tag="sims2")
    src = bass.AP(tensor=aq_all.tensor, offset=128,
                  ap=[[H_ * W2L - 1, 128], [W2L, H_], [1, L_]])
    dst = bass.AP(tensor=sims2.tensor, offset=0,
                  ap=[[H_ * L_, 128], [L_, H_], [1, L_]])
    nc.sync.dma_start(out=dst, in_=src)
    s3t = sb1.tile([128, H_, L_], fp32, tag="s3t")
    src = bass.AP(tensor=bk_all.tensor, offset=128,
                  ap=[[H_ * W2L - 1, 128], [W2L, H_], [1, L_]])
    dst = bass.AP(tensor=s3t.tensor, offset=0,
                  ap=[[H_ * L_, 128], [L_, H_], [1, L_]])
    nc.sync.dma_start(out=dst, in_=src)

    # ---- pass 2: per-head sims assembly + softmax ----
    e_all = sb.tile([128, H_, L_], bf16, tag="eall")
    z_all = sb.tile([128, H_], fp32, tag="zall")
    for h in range(H_):
        r0 = (h % 2) * 64
        dcq, dck = h // 2, 4 + h // 2
        qTl = qk_sb[r0:r0 + 64, dcq, 0:128]
        kT = qk_sb[r0:r0 + 64, dck, :]
        ps_s = psS.tile([128, L_], fp32, tag="sims")
        # content
        nc.tensor.matmul(ps_s, qTl, kT, start=True, stop=False)
        # key-RPE m=128 column
        nc.tensor.matmul(ps_s[:, 128:129], ktrT[r0:r0 + 64, h, 0:128],
                         qk_sb[r0:r0 + 64, dck, 128:129],
                         start=False, stop=False)
        # key-RPE main block: transpose-accumulate s3t [m, l] -> [l, m]
        nc.tensor.matmul(ps_s[:, 0:128], s3t[:, h, 0:128], id_f32,
                         is_transpose=True, start=False, stop=True)
        # query-RPE add
        nc.vector.tensor_add(out=ps_s, in0=ps_s, in1=sims2[:, h, :])
        # exp + row sums
        nc.scalar.activation(out=e_all[:, h, :], in_=ps_s, func=Exp,
                             accum_out=z_all[:, h:h + 1])

    r_all = sb.tile([128, H_], fp32, tag="rall")
    nc.vector.reciprocal(out=r_all, in_=z_all)
    w_all = sb.tile([128, H_, L_], bf16, tag="wall")
    rb = bass.AP(tensor=r_all.tensor, offset=0,
                 ap=[[H_, 128], [1, H_], [0, L_]])
    nc.vector.tensor_mul(out=w_all, in0=e_all, in1=rb)
    # fp32 copy of the m=128 w column (tensor_scalar needs fp32 scalars)
    wcol32 = sb.tile([128, H_], fp32, tag="wcol32")
    nc.vector.tensor_mul(out=wcol32, in0=e_all[:, :, 128], in1=r_all)


    # ---- l=128 remainder rows (per head, single partition) ----
    for h in range(H_):
        r0 = (h % 2) * 64
        dcq, dck = h // 2, 4 + h // 2
        ps_row = psR.tile([1, L_], fp32, tag="row")
        nc.tensor.matmul(ps_row, qk_sb[r0:r0 + 64, dcq, 128:129],
                         qk_sb[r0:r0 + 64, dck, :], start=True, stop=False)
        nc.tensor.matmul(ps_row, qk_sb[r0:r0 + 64, dcq, 128:129],
                         qtT[r0:r0 + 64, h, 0:L_], start=False, stop=False)
        tmpk = sb3.tile([128, L_], bf16, tag="tmpk")
        nc.vector.tensor_mul(out=tmpk[r0:r0 + 64, :],
                             in0=qk_sb[r0:r0 + 64, dck, :],
                             in1=ktnT[r0:r0 + 64, h, :])
        nc.tensor.matmul(ps_row, ones_sb[r0:r0 + 64, 0:1],
                         tmpk[r0:r0 + 64, :], start=False, stop=True)
        e1 = tiny.tile([1, L_], fp32, tag="e1")
        z1 = tiny.tile([1, 1], fp32, tag="z1")
        nc.scalar.activation(out=e1, in_=ps_row, func=Exp, accum_out=z1)
        r1 = tiny.tile([1, 1], fp32, tag="r1")
        nc.vector.reciprocal(out=r1, in_=z1)
        w1h = tiny.tile([1, L_], bf16, tag=f"w1h{h}")
        nc.vector.tensor_scalar_mul(out=w1h, in0=e1, scalar1=r1)
        w1s = tiny.tile([1, 1], fp32, tag=f"w1s{h}")
        nc.vector.tensor_scalar_mul(out=w1s, in0=e1[0:1, 128:129],
                                    scalar1=r1)
        w1h_list.append((w1h, w1s))
        ptw1 = psT.tile([128, 128], bf16, tag="ptr")
        nc.tensor.matmul(ptw1[:, 0:1], w1h[0:1, 0:128], id_bf[0:1, 0:1],
                         is_transpose=True, start=True, stop=True)
        nc.vector.tensor_copy(out=w1t_sb[:, h:h + 1], in_=ptw1[:, 0:1])

    # ---- W2 scatter ----
    w2_all = sb1.tile([128, H_, W2L], bf16, tag="w2")
    nc.gpsimd.memset(w2_all, 0.0)
    src = bass.AP(tensor=w_all.tensor, offset=0,
                  ap=[[H_ * L_, 128], [L_, H_], [1, 128]])
    dst = bass.AP(tensor=w2_all.tensor, offset=128,
                  ap=[[H_ * W2L - 1, 128], [W2L, H_], [1, 128]])
    nc.sync.dma_start(out=dst, in_=src)

    # ---- retrievals + output ----
    out_sb = sb.tile([128, H_, DV_], fp32, tag="outsb")
    o1_sb = sb.tile([1, 1024], fp32, tag="o1sb")
    for h in range(H_):
        # transposes: w -> wT, W2 chunks -> W2T
        ptw = psT.tile([128, 128], bf16, tag="ptr")
        nc.tensor.matmul(ptw, w_all[:, h, 0:128], id_bf,
                         is_transpose=True, start=True, stop=True)
        wt = sb3.tile([128, 128], bf16, tag="wt")
        nc.vector.tensor_copy(out=wt, in_=ptw)
        pt0 = psT.tile([128, 128], bf16, tag="ptr")
        nc.tensor.matmul(pt0, w2_all[:, h, 0:128], id_bf,
                         is_transpose=True, start=True, stop=True)
        w2t0 = sb3.tile([128, 128], bf16, tag="w2t0")
        nc.vector.tensor_copy(out=w2t0, in_=pt0)
        pt1 = psT.tile([128, 128], bf16, tag="ptr")
        nc.tensor.matmul(pt1, w2_all[:, h, 128:256], id_bf,
                         is_transpose=True, start=True, stop=True)
        w2t1 = sb3.tile([128, 128], bf16, tag="w2t1")
        nc.vector.tensor_copy(out=w2t1, in_=pt1)

        # ret1 m=128 rank-1 term: transpose w column -> row, K=1 matmul
        ps_wc = psT.tile([128, 128], bf16, tag="ptr")
        nc.tensor.matmul(ps_wc[0:1, :], w_all[:, h, 128:129], id_bf,
                         is_transpose=True, start=True, stop=True)
        wc_sb = tiny.tile([1, 128], bf16, tag="wc")
        nc.vector.tensor_copy(out=wc_sb, in_=ps_wc[0:1, :])

        ps_r = psR.tile([128, DV_], fp32, tag="ret")
        nc.tensor.matmul(ps_r, wt, v_sb[:, h, :], start=True, stop=False)
        nc.tensor.matmul(ps_r, w2t0, vt0[:, h, :], start=False, stop=False)
        nc.tensor.matmul(ps_r, w2t1, vt1[:, h, :], start=False, stop=False)
        nc.tensor.matmul(ps_r, wc_sb, v1b[0:1, h, :], start=False, stop=True)
        # m=128 value-RPE term: w[l, 128] * vt[256 - l, :]
        tmpv = sb3.tile([128, DV_], fp32, tag="tmpv")
        nc.vector.tensor_scalar_mul(out=tmpv, in0=vtrev[:, h, :],
                                    scalar1=wcol32[:, h:h + 1])
        nc.vector.tensor_add(out=ps_r, in0=ps_r, in1=tmpv)
        nc.vector.tensor_add(
            out=out_sb[:, h, :], in0=ps_r,
            in1=tbb[:, h * DV_:(h + 1) * DV_])

        # l=128 output row
        w1h, w1s = w1h_list[h]
        ps_r1 = psR.tile([1, L_], fp32, tag="row")
        nc.tensor.matmul(ps_r1[0:1, 0:DV_], w1t_sb[:, h:h + 1],
                         v_sb[:, h, :], start=True, stop=False)
        nc.tensor.matmul(ps_r1[0:1, 0:DV_], w1t_sb[:, h:h + 1],
                         vt0[:, h, :], start=False, stop=True)
        vv1 = tiny.tile([1, DV_], fp32, tag="vv1")
        nc.vector.tensor_add(out=vv1, in0=vt1[0:1, h, :], in1=v1b[0:1, h, :])
        tmp1 = tiny.tile([1, DV_], fp32, tag="tmp1")
        nc.vector.tensor_scalar_mul(out=tmp1, in0=vv1, scalar1=w1s)
        nc.vector.tensor_add(out=ps_r1[0:1, 0:DV_], in0=ps_r1[0:1, 0:DV_],
                             in1=tmp1)
        nc.vector.tensor_add(out=o1_sb[0:1, h * DV_:(h + 1) * DV_],
                             in0=ps_r1[0:1, 0:DV_],
                             in1=tb_sb[0:1, h * DV_:(h + 1) * DV_])

    nc.sync.dma_start(out=OUT[b, 0:128, :],
                      in_=out_sb.rearrange("p a b -> p (a b)"))
    nc.sync.dma_start(out=OUT[b, 128:129, :], in_=o1_sb)


def _get_program(nb):
    if nb not in _PROGRAM_CACHE:
        _PROGRAM_CACHE[nb] = _build_program(nb)
    return _PROGRAM_CACHE[nb]


LAST_HW_EXEC_NS = None


def kernel(**inputs):
    global LAST_HW_EXEC_NS
    from concourse.bass_utils import run_bass_kernel_spmd

    arrs = _host_prep(inputs)
    x = np.asarray(inputs["input_tensor"], np.float32)
    xpad = np.zeros((NCORES * NB, L, C), np.float32)
    xpad[:B] = x
    xbf = xpad.astype(BF16).reshape(NCORES, NB, L, C)

    nc = _get_program(NB)
    in_maps = []
    for c in range(NCORES):
        m = {"x": xbf[c]}
        m.update(arrs)
        in_maps.append(m)
    trace = bool(int(os.environ.get("BASS_KERNEL_TRACE", "0")))
    res = run_bass_kernel_spmd(nc, in_maps, list(range(NCORES)), trace=trace)
    if res.exec_time_ns is not None:
        LAST_HW_EXEC_NS = int(res.exec_time_ns)
    out = np.concatenate([res.results[c]["out"] for c in range(NCORES)],
                         axis=0)[:B]
    return np.ascontiguousarray(out, dtype=np.float32)
